# revision 14
# baseline (speedup 1.0000x reference)
"""Causal depthwise-conv MLP block (input proj -> causal depthwise conv1d ->
SiLU -> output proj) on 8 Trainium2 NeuronCores.

Sharding: sequence-parallel. B*S = 16384 tokens are split into 8 contiguous
shards of 2048 tokens (each batch of 4096 tokens spans exactly 2 cores). The
causal conv halo (3 tokens) is materialized host-side: each core's x tile
carries 3 leading halo columns whose values (the input projection of the 3
tokens preceding the shard, zeros at sequence starts) are precomputed on the
host, so no collectives are needed.

Device layout: channels on partitions, tokens on the free dim. All transposes
are done on the host (free): the kernel consumes hidden^T tiles and pre-tiled
transposed weights, and produces out^T, which the host transposes back.

Each core processes its 2048 tokens in 2 half-passes of 1024 tokens:
  phase 1: x[c,t] = w_in @ h^T + b_in  (bf16 matmuls, fp32 PSUM accum),
           written to SBUF as bf16 via DVE tensor_scalar_add
  phase 2: y = silu(depthwise_causal_conv(x) + conv_b), computed with 4
           shifted per-partition tensor_scalar muls + adds on DVE, SiLU on
           ScalarE, in-place over x
  phase 3: out[h,t] = w_out @ y + b_out (bf16 matmuls, fp32 PSUM) -> fp32 out

ht DRAM layout is block-contiguous: [half, blk, 128, n_k, 512] so each
(half, blk) loads as 4 DMAs of 4KB-per-partition contiguous chunks (full DMA
engine bandwidth; 1KB packets from the old column-split layout throttled the
startup to ~5x less per-op bandwidth and starved the first matmuls).
"""

import numpy as np
import ml_dtypes

BF16 = ml_dtypes.bfloat16

# full-size problem config
B, S, H, C, KSZ = 4, 4096, 2048, 4096, 4
N_CORES = 8
T_CORE = (B * S) // N_CORES      # tokens per core (2048)
N_HALF = 2
TH = T_CORE // N_HALF            # tokens per half-pass (1024)
BLK = 512                        # matmul N per PSUM bank (fp32 limit)
HALO = KSZ - 1                   # 3


def _build_module(cfg):
    """Emit the Bass/Tile module for one core (SPMD across all cores).

    cfg: dict with keys H, C, TH, BLK, n_half (token halves), used so a
    scaled-down config can be validated in CoreSim.
    """
    import concourse.bacc as bacc
    import concourse.mybir as mybir
    import concourse.tile as tile

    fp32 = mybir.dt.float32
    bf16 = mybir.dt.bfloat16
    AF = mybir.ActivationFunctionType

    cH, cC, cTH, cBLK, n_half = (
        cfg["H"], cfg["C"], cfg["TH"], cfg["BLK"], cfg["n_half"])
    # CoreSim doesn't implement Silu; cfg can swap in Sigmoid for sim tests
    act_fn = (AF.Sigmoid if cfg.get("act") == "sigmoid" else AF.Silu)
    n_k = cH // 128      # contraction tiles for input proj
    n_ct = cC // 128     # channel tiles
    n_ht = cH // 128     # output feature tiles
    n_kc = cC // 128     # contraction tiles for output proj
    n_blk = cTH // cBLK  # token blocks per half
    XW = HALO + cTH      # x columns per half
    KG = min(4, n_k)     # k-tiles per ht sub-DMA (4KB/partition chunks)
    n_g = n_k // KG

    nc = bacc.Bacc("TRN2", target_bir_lowering=False, debug=False,
                   num_devices=N_CORES)

    # block-contiguous hidden^T: [half, blk, 128, k, cols]
    ht_d = nc.dram_tensor("ht", [n_half, n_blk, 128, n_k, cBLK], bf16,
                          kind="ExternalInput")
    w_in_d = nc.dram_tensor("w_in_t", [n_ct, 128, n_k, 128], bf16,
                            kind="ExternalInput")
    w_out_d = nc.dram_tensor("w_out_t", [n_ht, 128, n_kc, 128], bf16,
                             kind="ExternalInput")
    b_in_d = nc.dram_tensor("b_in_c", [128, n_ct], fp32, kind="ExternalInput")
    # host-precomputed halo-x columns (projection of the 3 halo tokens per
    # half): 0.0015% of the FLOPs, kills all N=3 matmul chains on the PE
    xhalo_d = nc.dram_tensor("xhalo_c", [128, n_half, n_ct, HALO], bf16,
                             kind="ExternalInput")
    conv_w_d = nc.dram_tensor("conv_w_c", [128, n_ct, KSZ], fp32,
                              kind="ExternalInput")
    conv_b_d = nc.dram_tensor("conv_b_c", [128, n_ct], fp32,
                              kind="ExternalInput")
    b_out_d = nc.dram_tensor("b_out_c", [128, n_ht], fp32,
                             kind="ExternalInput")
    out_d = nc.dram_tensor("outt", [n_half, n_ht, 128, cTH], fp32,
                           kind="ExternalOutput")
    debug = cfg.get("debug", False)
    if debug:
        x_dbg = nc.dram_tensor("x_dbg", [n_half, n_ct, 128, XW], fp32,
                               kind="ExternalOutput")
        ya_dbg = nc.dram_tensor("ya_dbg", [n_half, n_ct, 128, XW], fp32,
                                kind="ExternalOutput")

    with tile.TileContext(nc) as tc:
        from contextlib import ExitStack
        with ExitStack() as ctx:
            consts = ctx.enter_context(tc.tile_pool(name="consts", bufs=1))
            ht_pool = ctx.enter_context(
                tc.tile_pool(name="ht", bufs=n_half * n_blk))
            x_pool = ctx.enter_context(tc.tile_pool(name="x", bufs=n_ct + 2))
            win_pool = ctx.enter_context(tc.tile_pool(name="win", bufs=6))
            wout_pool = ctx.enter_context(tc.tile_pool(name="wout", bufs=3))
            tmp_pool = ctx.enter_context(tc.tile_pool(name="tmp", bufs=2))
            out_pool = ctx.enter_context(tc.tile_pool(name="outp", bufs=4))
            ps_ab = ctx.enter_context(
                tc.tile_pool(name="ps_ab", bufs=2 * n_blk + 1, space="PSUM"))
            ps_out = ctx.enter_context(
                tc.tile_pool(name="ps_out", bufs=8 - 2 * n_blk - 1,
                             space="PSUM"))

            # first weight tile ahead of everything on the sync ring: the
            # very first matmul gates on it. Split so the k=0..3 slices land
            # first and the first matmul starts sooner.
            w0_sb = win_pool.tile([128, n_k, 128], bf16, tag="win",
                                  name="w0_sb")
            k0 = min(4, n_k)
            nc.sync.dma_start(out=w0_sb[:, 0:k0, :],
                              in_=w_in_d[0, :, 0:k0, :])
            if k0 < n_k:
                nc.sync.dma_start(out=w0_sb[:, k0:n_k, :],
                                  in_=w_in_d[0, :, k0:n_k, :])

            # all ht blocks for both halves, issued up front on the ACT ring
            # in consumption order (the SP ring carries the weights — ht on
            # it would queue ahead of w1..w3 in the ring FIFO and starve the
            # PE). Each sub-DMA moves KG k-tiles whose per-partition bytes
            # are contiguous in DRAM (KG*cBLK*2 = 4KB).
            ht_tiles = {}
            for half in range(n_half):
                for b in range(n_blk):
                    t = ht_pool.tile([128, n_k, cBLK], bf16, tag="ht",
                                     name=f"ht_{half}_{b}")
                    ht_tiles[(half, b)] = t
                    if half == 0 and b == 0 and n_k >= 16:
                        # very first piece is one k-tile (0.13MB) so the
                        # first matmul's gate opens ~1.7us earlier
                        splits = [(0, 1), (1, 2), (2, 4), (4, 8),
                                  (8, 12), (12, 16)]
                    else:
                        splits = [(g * KG, (g + 1) * KG) for g in range(n_g)]
                    for lo, hi in splits:
                        nc.scalar.dma_start(
                            out=t[:, lo:hi, :],
                            in_=ht_d[half, b, :, lo:hi, :])

            # next weight tiles ahead of the consts on the sync ring: the
            # PRE channel tiles need w1..w3 by ~14us, the consts later.
            pre_w = {}
            PRE = min(6, n_ct) if n_blk > 1 else 0
            for ct in range(1, PRE):
                w_sb = win_pool.tile([128, n_k, 128], bf16, tag="win",
                                     name="w_sb")
                nc.sync.dma_start(out=w_sb[:, :, :], in_=w_in_d[ct, :, :, :])
                pre_w[ct] = w_sb

            b_in_sb = consts.tile([128, n_ct], fp32)
            nc.sync.dma_start(out=b_in_sb[:, :], in_=b_in_d[:, :])
            xhalo_sb = consts.tile([128, n_half, n_ct, HALO], bf16)
            nc.sync.dma_start(out=xhalo_sb[:, :, :, :],
                              in_=xhalo_d[:, :, :, :])
            cw_sb = consts.tile([128, n_ct, KSZ], fp32)
            nc.sync.dma_start(out=cw_sb[:, :, :], in_=conv_w_d[:, :, :])
            cb_sb = consts.tile([128, n_ct], fp32)
            nc.sync.dma_start(out=cb_sb[:, :], in_=conv_b_d[:, :])
            b_out_sb = consts.tile([128, n_ht], fp32)
            nc.sync.dma_start(out=b_out_sb[:, :], in_=b_out_d[:, :])

            for half in range(n_half):
                ht_at = lambda k, b: ht_tiles[(half, b)][:, k, :]

                def p1_weights(ct):
                    if half == 0 and ct == 0:
                        return w0_sb
                    if half == 0 and ct in pre_w:
                        return pre_w[ct]
                    w_sb = win_pool.tile([128, n_k, 128], bf16,
                                         tag="win", name="w_sb")
                    nc.sync.dma_start(out=w_sb[:, :, :],
                                      in_=w_in_d[ct, :, :, :])
                    return w_sb

                def p1_mm(ct, w_sb, psum, b):
                    for k in range(n_k):
                        nc.tensor.matmul(
                            out=psum[:, :], lhsT=w_sb[:, k, :],
                            rhs=ht_at(k, b),
                            start=(k == 0), stop=(k == n_k - 1))

                def p1_act(ct, x_sb, psum, b):
                    # on DVE, not ScalarE: the ACT sequencer is busy issuing
                    # ht DMAs at startup, which delayed PSUM slot releases
                    nc.vector.tensor_scalar_add(
                        x_sb[:, HALO + b * cBLK:HALO + (b + 1) * cBLK],
                        psum[:, :], b_in_sb[:, ct:ct + 1])

                def p1_halo(ct, x_sb):
                    nc.vector.tensor_copy(x_sb[:, 0:HALO],
                                          xhalo_sb[:, half, ct, :])

                def p2_conv(ct, x_sb):
                    if debug:
                        xdf = tmp_pool.tile([128, XW], fp32, tag="xdf",
                                            name="xdf")
                        nc.vector.tensor_copy(xdf[:, :], x_sb[:, :])
                        nc.sync.dma_start(out=x_dbg[half, ct, :, :],
                                          in_=xdf[:, :])
                    # conv + silu for this channel tile, in-place over x.
                    # Blocks in descending t order so the in-place write
                    # never clobbers columns a later block still needs.
                    for b in reversed(range(n_blk)):
                        t0 = b * cBLK
                        m0 = tmp_pool.tile([128, cBLK], bf16, tag="m0",
                                           name="m0")
                        nc.vector.tensor_scalar_mul(
                            m0[:, :], x_sb[:, t0:t0 + cBLK],
                            cw_sb[:, ct, 0:1])
                        m1 = tmp_pool.tile([128, cBLK], bf16, tag="m1",
                                           name="m1")
                        nc.vector.tensor_scalar_mul(
                            m1[:, :], x_sb[:, t0 + 1:t0 + 1 + cBLK],
                            cw_sb[:, ct, 1:2])
                        nc.vector.tensor_add(m0[:, :], m0[:, :], m1[:, :])
                        m2 = tmp_pool.tile([128, cBLK], bf16, tag="m2",
                                           name="m2")
                        nc.vector.tensor_scalar_mul(
                            m2[:, :], x_sb[:, t0 + 2:t0 + 2 + cBLK],
                            cw_sb[:, ct, 2:3])
                        m3 = tmp_pool.tile([128, cBLK], bf16, tag="m3",
                                           name="m3")
                        nc.vector.tensor_scalar_mul(
                            m3[:, :], x_sb[:, t0 + 3:t0 + 3 + cBLK],
                            cw_sb[:, ct, 3:4])
                        nc.vector.tensor_add(m2[:, :], m2[:, :], m3[:, :])
                        nc.vector.tensor_add(m0[:, :], m0[:, :], m2[:, :])
                        nc.scalar.activation(
                            out=x_sb[:, HALO + t0:HALO + t0 + cBLK],
                            in_=m0[:, :], func=act_fn,
                            bias=cb_sb[:, ct:ct + 1])

                x_tiles = []
                # prefix: emit block-0 matmuls of the first PRE channel tiles
                # back-to-back so the PE has work while block-1 columns load
                pend = []
                for ct in range(PRE if half == 0 else 0):
                    w_sb = p1_weights(ct)
                    x_sb = x_pool.tile([128, XW], bf16, tag="x", name="x_sb")
                    p0 = ps_ab.tile([128, cBLK], fp32, tag="pab", name="pab0")
                    p1_mm(ct, w_sb, p0, 0)
                    p1_act(ct, x_sb, p0, 0)
                    pend.append((ct, w_sb, x_sb))
                for ct, w_sb, x_sb in pend:
                    for b in range(1, n_blk):
                        pb = ps_ab.tile([128, cBLK], fp32, tag="pab",
                                        name=f"pab{b}")
                        p1_mm(ct, w_sb, pb, b)
                        p1_act(ct, x_sb, pb, b)
                    p1_halo(ct, x_sb)
                    x_tiles.append(x_sb)
                    p2_conv(ct, x_sb)
                for ct in range(len(pend), n_ct):
                    w_sb = p1_weights(ct)
                    x_sb = x_pool.tile([128, XW], bf16, tag="x", name="x_sb")
                    for b in range(n_blk):
                        pb = ps_ab.tile([128, cBLK], fp32, tag="pab",
                                        name=f"pab{b}")
                        p1_mm(ct, w_sb, pb, b)
                        p1_act(ct, x_sb, pb, b)
                    p1_halo(ct, x_sb)
                    x_tiles.append(x_sb)
                    p2_conv(ct, x_sb)

                if debug:
                    for ct in range(n_ct):
                        ydf = tmp_pool.tile([128, XW], fp32, tag="ydf",
                                            name="ydf")
                        nc.vector.tensor_copy(ydf[:, :], x_tiles[ct][:, :])
                        nc.sync.dma_start(out=ya_dbg[half, ct, :, :],
                                          in_=ydf[:, :])

                # phase 3: output projection over all channel tiles
                for ht_i in range(n_ht):
                    wo_sb = wout_pool.tile([128, n_kc, 128], bf16, tag="wout")
                    nc.sync.dma_start(out=wo_sb[:, :, :],
                                      in_=w_out_d[ht_i, :, :, :])
                    po = [ps_out.tile([128, cBLK], fp32, tag="po",
                                      name=f"po{b}")
                          for b in range(n_blk)]
                    for kc in range(n_kc):
                        st, sp = (kc == 0), (kc == n_kc - 1)
                        for b in range(n_blk):
                            nc.tensor.matmul(
                                out=po[b][:, :], lhsT=wo_sb[:, kc, :],
                                rhs=x_tiles[kc][:,
                                                HALO + b * cBLK:HALO + (b + 1) * cBLK],
                                start=st, stop=sp)
                    for b in range(n_blk):
                        ob = out_pool.tile([128, cBLK], fp32, tag="ob")
                        nc.scalar.activation(
                            out=ob[:, :], in_=po[b][:, :], func=AF.Identity,
                            bias=b_out_sb[:, ht_i:ht_i + 1])
                        nc.scalar.dma_start(
                            out=out_d[half, ht_i, :, b * cBLK:(b + 1) * cBLK],
                            in_=ob[:, :])

    nc.compile()
    return nc


_MODULE_CACHE = {}


def _get_module(cfg_key, cfg):
    if cfg_key not in _MODULE_CACHE:
        _MODULE_CACHE[cfg_key] = _build_module(cfg)
    return _MODULE_CACHE[cfg_key]


def _pack_shared(w_in, b_in, conv_w, conv_b, w_out, b_out):
    """Host-side packing of the core-independent inputs."""
    n_k, n_ct = H // 128, C // 128
    n_ht, n_kc = H // 128, C // 128
    w_in_t = np.ascontiguousarray(
        w_in.T.astype(BF16).reshape(n_k, 128, n_ct, 128).transpose(2, 1, 0, 3))
    w_out_t = np.ascontiguousarray(
        w_out.T.astype(BF16).reshape(n_kc, 128, n_ht, 128).transpose(2, 1, 0, 3))
    b_in_c = np.ascontiguousarray(
        b_in.astype(np.float32).reshape(n_ct, 128).T)
    conv_w_c = np.ascontiguousarray(
        conv_w.reshape(C, KSZ).astype(np.float32)
        .reshape(n_ct, 128, KSZ).transpose(1, 0, 2))
    conv_b_c = np.ascontiguousarray(
        conv_b.astype(np.float32).reshape(n_ct, 128).T)
    b_out_c = np.ascontiguousarray(
        b_out.astype(np.float32).reshape(n_ht, 128).T)
    return {
        "w_in_t": w_in_t, "w_out_t": w_out_t, "b_in_c": b_in_c,
        "conv_w_c": conv_w_c, "conv_b_c": conv_b_c, "b_out_c": b_out_c,
    }


def _pack_core(ht_all, w_in_f, b_in, core):
    """Per-core hidden^T blocks and host-computed halo-x columns (projection
    of the 3 tokens preceding each half)."""
    n_k = H // 128
    n_ct = C // 128
    n_blk = TH // BLK
    ht_core = np.empty((N_HALF, n_blk, 128, n_k, BLK), dtype=BF16)
    xhalo = np.zeros((N_HALF, HALO, C), dtype=np.float32)
    for half in range(N_HALF):
        base = core * T_CORE + half * TH
        for b in range(n_blk):
            cols = ht_all[:, base + b * BLK:base + (b + 1) * BLK]
            ht_core[half, b] = cols.reshape(n_k, 128, BLK).transpose(1, 0, 2)
        if not (half == 0 and core % 2 == 0):
            h_halo = ht_all[:, base - HALO:base].astype(np.float32)  # [H, 3]
            xhalo[half] = h_halo.T @ w_in_f.T + b_in[None, :]
    # [half, j, ct*128+p] -> [p, half, ct, j]
    xhalo_c = np.ascontiguousarray(
        xhalo.reshape(N_HALF, HALO, n_ct, 128)
        .transpose(3, 0, 2, 1).astype(BF16))
    return {"ht": ht_core, "xhalo_c": xhalo_c}


def _ensure_axon_hooks():
    """concourse's trace path imports antenv.axon_hooks, which not every
    image ships. Register a stub (hook=None -> tracing skipped gracefully)
    so a BASS_TRACE=1 environment without it doesn't crash the run."""
    try:
        import antenv.axon_hooks  # noqa: F401
    except Exception:
        import sys
        import types
        mod = types.ModuleType("antenv.axon_hooks")
        mod._h = None
        mod.set_axon_ntff_profile_hook = lambda h: setattr(mod, "_h", h)
        mod.get_axon_ntff_profile_hook = lambda: mod._h
        sys.modules["antenv.axon_hooks"] = mod


def _run(hidden_states, w_in, b_in, conv_w, conv_b, w_out, b_out,
         trace=False):
    _ensure_axon_hooks()
    from concourse import bass_utils

    cfg = {"H": H, "C": C, "TH": TH, "BLK": BLK, "n_half": N_HALF}
    nc = _get_module("full", cfg)

    hidden = np.asarray(hidden_states, dtype=np.float32)
    ht_all = np.ascontiguousarray(
        hidden.reshape(B * S, H).astype(BF16).T)  # [H, B*S]

    shared = _pack_shared(np.asarray(w_in), np.asarray(b_in),
                          np.asarray(conv_w), np.asarray(conv_b),
                          np.asarray(w_out), np.asarray(b_out))
    w_in_f = np.asarray(w_in, dtype=np.float32)
    b_in_f = np.asarray(b_in, dtype=np.float32)
    in_maps = []
    for core in range(N_CORES):
        m = dict(shared)
        m.update(_pack_core(ht_all, w_in_f, b_in_f, core))
        in_maps.append(m)

    res = bass_utils.run_bass_kernel_spmd(
        nc, in_maps, core_ids=list(range(N_CORES)), trace=trace)

    out_full = np.empty((B * S, H), dtype=np.float32)
    for core in range(N_CORES):
        ot = res.results[core]["outt"]  # [n_half, n_ht, 128, TH]
        out_full[core * T_CORE:(core + 1) * T_CORE] = (
            ot.transpose(0, 3, 1, 2).reshape(T_CORE, H))
    return out_full.reshape(B, S, H), res


def kernel(hidden_states, w_in, b_in, conv_w, conv_b, w_out, b_out):
    return _run(hidden_states, w_in, b_in, conv_w, conv_b, w_out, b_out)[0]


# revision 16
# speedup vs baseline: 1.0020x; 1.0020x over previous
"""Causal depthwise-conv MLP block (input proj -> causal depthwise conv1d ->
SiLU -> output proj) on 8 Trainium2 NeuronCores.

Sharding: sequence-parallel. B*S = 16384 tokens are split into 8 contiguous
shards of 2048 tokens (each batch of 4096 tokens spans exactly 2 cores). The
causal conv halo (3 tokens) is materialized host-side: each core's x tile
carries 3 leading halo columns whose values (the input projection of the 3
tokens preceding the shard, zeros at sequence starts) are precomputed on the
host, so no collectives are needed.

Device layout: channels on partitions, tokens on the free dim. All transposes
are done on the host (free): the kernel consumes hidden^T tiles and pre-tiled
transposed weights, and produces out^T, which the host transposes back.

Each core processes its 2048 tokens in 2 half-passes of 1024 tokens:
  phase 1: x[c,t] = w_in @ h^T + b_in  (bf16 matmuls, fp32 PSUM accum),
           written to SBUF as bf16 via DVE tensor_scalar_add
  phase 2: y = silu(depthwise_causal_conv(x) + conv_b), computed with 4
           shifted per-partition tensor_scalar muls + adds on DVE, SiLU on
           ScalarE, in-place over x
  phase 3: out[h,t] = w_out @ y + b_out (bf16 matmuls, fp32 PSUM) -> fp32 out

ht DRAM layout is block-contiguous: [half, blk, 128, n_k, 512] so each
(half, blk) loads as 4 DMAs of 4KB-per-partition contiguous chunks (full DMA
engine bandwidth; 1KB packets from the old column-split layout throttled the
startup to ~5x less per-op bandwidth and starved the first matmuls).
"""

import numpy as np
import ml_dtypes

BF16 = ml_dtypes.bfloat16

# full-size problem config
B, S, H, C, KSZ = 4, 4096, 2048, 4096, 4
N_CORES = 8
T_CORE = (B * S) // N_CORES      # tokens per core (2048)
N_HALF = 2
TH = T_CORE // N_HALF            # tokens per half-pass (1024)
BLK = 512                        # matmul N per PSUM bank (fp32 limit)
HALO = KSZ - 1                   # 3


def _build_module(cfg):
    """Emit the Bass/Tile module for one core (SPMD across all cores).

    cfg: dict with keys H, C, TH, BLK, n_half (token halves), used so a
    scaled-down config can be validated in CoreSim.
    """
    import concourse.bacc as bacc
    import concourse.mybir as mybir
    import concourse.tile as tile

    fp32 = mybir.dt.float32
    bf16 = mybir.dt.bfloat16
    AF = mybir.ActivationFunctionType

    cH, cC, cTH, cBLK, n_half = (
        cfg["H"], cfg["C"], cfg["TH"], cfg["BLK"], cfg["n_half"])
    # CoreSim doesn't implement Silu; cfg can swap in Sigmoid for sim tests
    act_fn = (AF.Sigmoid if cfg.get("act") == "sigmoid" else AF.Silu)
    n_k = cH // 128      # contraction tiles for input proj
    n_ct = cC // 128     # channel tiles
    n_ht = cH // 128     # output feature tiles
    n_kc = cC // 128     # contraction tiles for output proj
    n_blk = cTH // cBLK  # token blocks per half
    XW = HALO + cTH      # x columns per half
    KG = min(4, n_k)     # k-tiles per ht sub-DMA (4KB/partition chunks)
    n_g = n_k // KG

    nc = bacc.Bacc("TRN2", target_bir_lowering=False, debug=False,
                   num_devices=N_CORES)

    # block-contiguous hidden^T: [half, blk, 128, k, cols]
    ht_d = nc.dram_tensor("ht", [n_half, n_blk, 128, n_k, cBLK], bf16,
                          kind="ExternalInput")
    w_in_d = nc.dram_tensor("w_in_t", [n_ct, 128, n_k, 128], bf16,
                            kind="ExternalInput")
    w_out_d = nc.dram_tensor("w_out_t", [n_ht, 128, n_kc, 128], bf16,
                             kind="ExternalInput")
    b_in_d = nc.dram_tensor("b_in_c", [128, n_ct], fp32, kind="ExternalInput")
    # host-precomputed halo-x columns (projection of the 3 halo tokens per
    # half): 0.0015% of the FLOPs, kills all N=3 matmul chains on the PE
    xhalo_d = nc.dram_tensor("xhalo_c", [128, n_half, n_ct, HALO], bf16,
                             kind="ExternalInput")
    conv_w_d = nc.dram_tensor("conv_w_c", [128, n_ct, KSZ], fp32,
                              kind="ExternalInput")
    conv_b_d = nc.dram_tensor("conv_b_c", [128, n_ct], fp32,
                              kind="ExternalInput")
    b_out_d = nc.dram_tensor("b_out_c", [128, n_ht], fp32,
                             kind="ExternalInput")
    out_d = nc.dram_tensor("outt", [n_half, n_ht, 128, cTH], fp32,
                           kind="ExternalOutput")
    debug = cfg.get("debug", False)
    if debug:
        x_dbg = nc.dram_tensor("x_dbg", [n_half, n_ct, 128, XW], fp32,
                               kind="ExternalOutput")
        ya_dbg = nc.dram_tensor("ya_dbg", [n_half, n_ct, 128, XW], fp32,
                                kind="ExternalOutput")

    with tile.TileContext(nc) as tc:
        from contextlib import ExitStack
        with ExitStack() as ctx:
            consts = ctx.enter_context(tc.tile_pool(name="consts", bufs=1))
            ht_pool = ctx.enter_context(
                tc.tile_pool(name="ht", bufs=n_half * n_blk))
            x_pool = ctx.enter_context(tc.tile_pool(name="x", bufs=n_ct + 2))
            win_pool = ctx.enter_context(tc.tile_pool(name="win", bufs=6))
            wout_pool = ctx.enter_context(tc.tile_pool(name="wout", bufs=3))
            tmp_pool = ctx.enter_context(tc.tile_pool(name="tmp", bufs=2))
            out_pool = ctx.enter_context(tc.tile_pool(name="outp", bufs=4))
            ps_ab = ctx.enter_context(
                tc.tile_pool(name="ps_ab", bufs=2 * n_blk + 1, space="PSUM"))
            ps_out = ctx.enter_context(
                tc.tile_pool(name="ps_out", bufs=8 - 2 * n_blk - 1,
                             space="PSUM"))

            # first weight tile ahead of everything on the sync ring: the
            # very first matmul gates on it. Split so the k=0..3 slices land
            # first and the first matmul starts sooner.
            w0_sb = win_pool.tile([128, n_k, 128], bf16, tag="win",
                                  name="w0_sb")
            k0 = min(4, n_k)
            nc.sync.dma_start(out=w0_sb[:, 0:k0, :],
                              in_=w_in_d[0, :, 0:k0, :])
            if k0 < n_k:
                nc.sync.dma_start(out=w0_sb[:, k0:n_k, :],
                                  in_=w_in_d[0, :, k0:n_k, :])

            # all ht blocks for both halves, issued up front on the ACT ring
            # in consumption order (the SP ring carries the weights — ht on
            # it would queue ahead of w1..w3 in the ring FIFO and starve the
            # PE). Each sub-DMA moves KG k-tiles whose per-partition bytes
            # are contiguous in DRAM (KG*cBLK*2 = 4KB).
            ht_tiles = {}
            for half in range(n_half):
                for b in range(n_blk):
                    t = ht_pool.tile([128, n_k, cBLK], bf16, tag="ht",
                                     name=f"ht_{half}_{b}")
                    ht_tiles[(half, b)] = t
                    for g in range(n_g):
                        nc.scalar.dma_start(
                            out=t[:, g * KG:(g + 1) * KG, :],
                            in_=ht_d[half, b, :, g * KG:(g + 1) * KG, :])

            # next weight tiles ahead of the consts on the sync ring: the
            # PRE channel tiles need w1..w3 by ~14us, the consts later.
            pre_w = {}
            PRE = min(4, n_ct) if n_blk > 1 else 0
            for ct in range(1, PRE):
                w_sb = win_pool.tile([128, n_k, 128], bf16, tag="win",
                                     name="w_sb")
                nc.sync.dma_start(out=w_sb[:, :, :], in_=w_in_d[ct, :, :, :])
                pre_w[ct] = w_sb

            b_in_sb = consts.tile([128, n_ct], fp32)
            nc.sync.dma_start(out=b_in_sb[:, :], in_=b_in_d[:, :])
            xhalo_sb = consts.tile([128, n_half, n_ct, HALO], bf16)
            nc.sync.dma_start(out=xhalo_sb[:, :, :, :],
                              in_=xhalo_d[:, :, :, :])
            cw_sb = consts.tile([128, n_ct, KSZ], fp32)
            nc.sync.dma_start(out=cw_sb[:, :, :], in_=conv_w_d[:, :, :])
            cb_sb = consts.tile([128, n_ct], fp32)
            nc.sync.dma_start(out=cb_sb[:, :], in_=conv_b_d[:, :])
            b_out_sb = consts.tile([128, n_ht], fp32)
            nc.sync.dma_start(out=b_out_sb[:, :], in_=b_out_d[:, :])

            for half in range(n_half):
                ht_at = lambda k, b: ht_tiles[(half, b)][:, k, :]

                def p1_weights(ct):
                    if half == 0 and ct == 0:
                        return w0_sb
                    if half == 0 and ct in pre_w:
                        return pre_w[ct]
                    w_sb = win_pool.tile([128, n_k, 128], bf16,
                                         tag="win", name="w_sb")
                    nc.sync.dma_start(out=w_sb[:, :, :],
                                      in_=w_in_d[ct, :, :, :])
                    return w_sb

                def p1_mm(ct, w_sb, psum, b):
                    for k in range(n_k):
                        nc.tensor.matmul(
                            out=psum[:, :], lhsT=w_sb[:, k, :],
                            rhs=ht_at(k, b),
                            start=(k == 0), stop=(k == n_k - 1))

                def p1_act(ct, x_sb, psum, b):
                    # on DVE, not ScalarE: the ACT sequencer is busy issuing
                    # ht DMAs at startup, which delayed PSUM slot releases
                    nc.vector.tensor_scalar_add(
                        x_sb[:, HALO + b * cBLK:HALO + (b + 1) * cBLK],
                        psum[:, :], b_in_sb[:, ct:ct + 1])

                def p1_halo(ct, x_sb):
                    nc.vector.tensor_copy(x_sb[:, 0:HALO],
                                          xhalo_sb[:, half, ct, :])

                def p2_conv(ct, x_sb):
                    if debug:
                        xdf = tmp_pool.tile([128, XW], fp32, tag="xdf",
                                            name="xdf")
                        nc.vector.tensor_copy(xdf[:, :], x_sb[:, :])
                        nc.sync.dma_start(out=x_dbg[half, ct, :, :],
                                          in_=xdf[:, :])
                    # conv + silu for this channel tile, in-place over x.
                    # Blocks in descending t order so the in-place write
                    # never clobbers columns a later block still needs.
                    for b in reversed(range(n_blk)):
                        t0 = b * cBLK
                        m0 = tmp_pool.tile([128, cBLK], bf16, tag="m0",
                                           name="m0")
                        nc.vector.tensor_scalar_mul(
                            m0[:, :], x_sb[:, t0:t0 + cBLK],
                            cw_sb[:, ct, 0:1])
                        m1 = tmp_pool.tile([128, cBLK], bf16, tag="m1",
                                           name="m1")
                        nc.vector.tensor_scalar_mul(
                            m1[:, :], x_sb[:, t0 + 1:t0 + 1 + cBLK],
                            cw_sb[:, ct, 1:2])
                        nc.vector.tensor_add(m0[:, :], m0[:, :], m1[:, :])
                        m2 = tmp_pool.tile([128, cBLK], bf16, tag="m2",
                                           name="m2")
                        nc.vector.tensor_scalar_mul(
                            m2[:, :], x_sb[:, t0 + 2:t0 + 2 + cBLK],
                            cw_sb[:, ct, 2:3])
                        m3 = tmp_pool.tile([128, cBLK], bf16, tag="m3",
                                           name="m3")
                        nc.vector.tensor_scalar_mul(
                            m3[:, :], x_sb[:, t0 + 3:t0 + 3 + cBLK],
                            cw_sb[:, ct, 3:4])
                        nc.vector.tensor_add(m2[:, :], m2[:, :], m3[:, :])
                        nc.vector.tensor_add(m0[:, :], m0[:, :], m2[:, :])
                        nc.scalar.activation(
                            out=x_sb[:, HALO + t0:HALO + t0 + cBLK],
                            in_=m0[:, :], func=act_fn,
                            bias=cb_sb[:, ct:ct + 1])

                x_tiles = []
                # prefix: emit block-0 matmuls of the first PRE channel tiles
                # back-to-back so the PE has work while block-1 columns load
                pend = []
                for ct in range(PRE if half == 0 else 0):
                    w_sb = p1_weights(ct)
                    x_sb = x_pool.tile([128, XW], bf16, tag="x", name="x_sb")
                    p0 = ps_ab.tile([128, cBLK], fp32, tag="pab", name="pab0")
                    p1_mm(ct, w_sb, p0, 0)
                    p1_act(ct, x_sb, p0, 0)
                    pend.append((ct, w_sb, x_sb))
                for ct, w_sb, x_sb in pend:
                    for b in range(1, n_blk):
                        pb = ps_ab.tile([128, cBLK], fp32, tag="pab",
                                        name=f"pab{b}")
                        p1_mm(ct, w_sb, pb, b)
                        p1_act(ct, x_sb, pb, b)
                    p1_halo(ct, x_sb)
                    x_tiles.append(x_sb)
                    p2_conv(ct, x_sb)
                for ct in range(len(pend), n_ct):
                    w_sb = p1_weights(ct)
                    x_sb = x_pool.tile([128, XW], bf16, tag="x", name="x_sb")
                    for b in range(n_blk):
                        pb = ps_ab.tile([128, cBLK], fp32, tag="pab",
                                        name=f"pab{b}")
                        p1_mm(ct, w_sb, pb, b)
                        p1_act(ct, x_sb, pb, b)
                    p1_halo(ct, x_sb)
                    x_tiles.append(x_sb)
                    p2_conv(ct, x_sb)

                if debug:
                    for ct in range(n_ct):
                        ydf = tmp_pool.tile([128, XW], fp32, tag="ydf",
                                            name="ydf")
                        nc.vector.tensor_copy(ydf[:, :], x_tiles[ct][:, :])
                        nc.sync.dma_start(out=ya_dbg[half, ct, :, :],
                                          in_=ydf[:, :])

                # phase 3: output projection over all channel tiles
                for ht_i in range(n_ht):
                    wo_sb = wout_pool.tile([128, n_kc, 128], bf16, tag="wout")
                    nc.sync.dma_start(out=wo_sb[:, :, :],
                                      in_=w_out_d[ht_i, :, :, :])
                    po = [ps_out.tile([128, cBLK], fp32, tag="po",
                                      name=f"po{b}")
                          for b in range(n_blk)]
                    for kc in range(n_kc):
                        st, sp = (kc == 0), (kc == n_kc - 1)
                        for b in range(n_blk):
                            nc.tensor.matmul(
                                out=po[b][:, :], lhsT=wo_sb[:, kc, :],
                                rhs=x_tiles[kc][:,
                                                HALO + b * cBLK:HALO + (b + 1) * cBLK],
                                start=st, stop=sp)
                    for b in range(n_blk):
                        ob = out_pool.tile([128, cBLK], fp32, tag="ob")
                        nc.scalar.activation(
                            out=ob[:, :], in_=po[b][:, :], func=AF.Identity,
                            bias=b_out_sb[:, ht_i:ht_i + 1])
                        nc.scalar.dma_start(
                            out=out_d[half, ht_i, :, b * cBLK:(b + 1) * cBLK],
                            in_=ob[:, :])

    nc.compile()
    return nc


_MODULE_CACHE = {}


def _get_module(cfg_key, cfg):
    if cfg_key not in _MODULE_CACHE:
        _MODULE_CACHE[cfg_key] = _build_module(cfg)
    return _MODULE_CACHE[cfg_key]


def _pack_shared(w_in, b_in, conv_w, conv_b, w_out, b_out):
    """Host-side packing of the core-independent inputs."""
    n_k, n_ct = H // 128, C // 128
    n_ht, n_kc = H // 128, C // 128
    w_in_t = np.ascontiguousarray(
        w_in.T.astype(BF16).reshape(n_k, 128, n_ct, 128).transpose(2, 1, 0, 3))
    w_out_t = np.ascontiguousarray(
        w_out.T.astype(BF16).reshape(n_kc, 128, n_ht, 128).transpose(2, 1, 0, 3))
    b_in_c = np.ascontiguousarray(
        b_in.astype(np.float32).reshape(n_ct, 128).T)
    conv_w_c = np.ascontiguousarray(
        conv_w.reshape(C, KSZ).astype(np.float32)
        .reshape(n_ct, 128, KSZ).transpose(1, 0, 2))
    conv_b_c = np.ascontiguousarray(
        conv_b.astype(np.float32).reshape(n_ct, 128).T)
    b_out_c = np.ascontiguousarray(
        b_out.astype(np.float32).reshape(n_ht, 128).T)
    return {
        "w_in_t": w_in_t, "w_out_t": w_out_t, "b_in_c": b_in_c,
        "conv_w_c": conv_w_c, "conv_b_c": conv_b_c, "b_out_c": b_out_c,
    }


def _pack_core(ht_all, w_in_f, b_in, core):
    """Per-core hidden^T blocks and host-computed halo-x columns (projection
    of the 3 tokens preceding each half)."""
    n_k = H // 128
    n_ct = C // 128
    n_blk = TH // BLK
    ht_core = np.empty((N_HALF, n_blk, 128, n_k, BLK), dtype=BF16)
    xhalo = np.zeros((N_HALF, HALO, C), dtype=np.float32)
    for half in range(N_HALF):
        base = core * T_CORE + half * TH
        for b in range(n_blk):
            cols = ht_all[:, base + b * BLK:base + (b + 1) * BLK]
            ht_core[half, b] = cols.reshape(n_k, 128, BLK).transpose(1, 0, 2)
        if not (half == 0 and core % 2 == 0):
            h_halo = ht_all[:, base - HALO:base].astype(np.float32)  # [H, 3]
            xhalo[half] = h_halo.T @ w_in_f.T + b_in[None, :]
    # [half, j, ct*128+p] -> [p, half, ct, j]
    xhalo_c = np.ascontiguousarray(
        xhalo.reshape(N_HALF, HALO, n_ct, 128)
        .transpose(3, 0, 2, 1).astype(BF16))
    return {"ht": ht_core, "xhalo_c": xhalo_c}


def _ensure_axon_hooks():
    """concourse's trace path imports antenv.axon_hooks, which not every
    image ships. Register a stub (hook=None -> tracing skipped gracefully)
    so a BASS_TRACE=1 environment without it doesn't crash the run."""
    try:
        import antenv.axon_hooks  # noqa: F401
    except Exception:
        import sys
        import types
        mod = types.ModuleType("antenv.axon_hooks")
        mod._h = None
        mod.set_axon_ntff_profile_hook = lambda h: setattr(mod, "_h", h)
        mod.get_axon_ntff_profile_hook = lambda: mod._h
        sys.modules["antenv.axon_hooks"] = mod


def _run(hidden_states, w_in, b_in, conv_w, conv_b, w_out, b_out,
         trace=False):
    _ensure_axon_hooks()
    from concourse import bass_utils

    cfg = {"H": H, "C": C, "TH": TH, "BLK": BLK, "n_half": N_HALF}
    nc = _get_module("full", cfg)

    hidden = np.asarray(hidden_states, dtype=np.float32)
    ht_all = np.ascontiguousarray(
        hidden.reshape(B * S, H).astype(BF16).T)  # [H, B*S]

    shared = _pack_shared(np.asarray(w_in), np.asarray(b_in),
                          np.asarray(conv_w), np.asarray(conv_b),
                          np.asarray(w_out), np.asarray(b_out))
    w_in_f = np.asarray(w_in, dtype=np.float32)
    b_in_f = np.asarray(b_in, dtype=np.float32)
    in_maps = []
    for core in range(N_CORES):
        m = dict(shared)
        m.update(_pack_core(ht_all, w_in_f, b_in_f, core))
        in_maps.append(m)

    res = bass_utils.run_bass_kernel_spmd(
        nc, in_maps, core_ids=list(range(N_CORES)), trace=trace)

    out_full = np.empty((B * S, H), dtype=np.float32)
    for core in range(N_CORES):
        ot = res.results[core]["outt"]  # [n_half, n_ht, 128, TH]
        out_full[core * T_CORE:(core + 1) * T_CORE] = (
            ot.transpose(0, 3, 1, 2).reshape(T_CORE, H))
    return out_full.reshape(B, S, H), res


def kernel(hidden_states, w_in, b_in, conv_w, conv_b, w_out, b_out):
    return _run(hidden_states, w_in, b_in, conv_w, conv_b, w_out, b_out)[0]
